# revision 9
# baseline (speedup 1.0000x reference)
"""CrossModalAttention Trainium2 kernel.

Per-core computation (data-parallel over batch, 1 sample per core):
  eeg_proj   = eeg @ W_e + b_e                  [T, U]
  image_proj = image @ W_i + b_i                [T, U]
  scores     = eeg_proj @ image_proj.T          [T, T]
  attn       = softmax(scores, axis=-1)
  att_eeg    = attn @ eeg_proj                  [T, U]
  att_img    = attn.T @ image_proj              [T, U]

Engine split: PE does only real matmuls (proj accumulation in f32r with
fp32 PSUM, scores + AV in fp16) plus the x-transposes (f32r, 1.5 c/row).
All other transposes run on the DMA XBAR (16x128 tile transpose):
projT->proj and the per-row-block E^T needed for att_eeg. Softmax max on
DVE, exp on ACT (exact per-row max subtraction), fp16 attention weights.
End-to-end absmax relative error ~5e-3.
"""
import numpy as np
from contextlib import ExitStack

import concourse.bass as bass
import concourse.bacc as bacc
import concourse.mybir as mybir
import concourse.tile as tile
from concourse.bass_utils import run_bass_kernel_spmd
from concourse.masks import make_identity

F32 = mybir.dt.float32
F32R = mybir.dt.float32r
F16 = mybir.dt.float16
AX = mybir.AxisListType.X
EXP = mybir.ActivationFunctionType.Exp
COPY = mybir.ActivationFunctionType.Copy

B, T, DE, DI, U = 8, 2048, 512, 768, 256
NCORES = 8
TQ = T // 128           # 16 q/k tiles of 128
NK = T // 512           # 4 score chunks of 512
NSTRIP = T // 512       # 4 strips of 512 rows per modality
AV_LAG = 2              # AV-eeg trails scores by 2 q-tiles (XBAR latency)


def _phase1_modality(nc, pools, x_dram, D, w_sb, b_col, projT, first):
    """Load x strips, PE-transpose to xT (f32r), project, bias->fp16 projT."""
    ps, psb, xstrip, xt, identr = pools
    NDC = D // 128
    x = x_dram.ap().bitcast(F32R)
    xT = [xt.tile([128, T], F32R, tag="xt", name=f"xT_{x_dram.name}_{dc}")
          for dc in range(NDC)]
    for s in range(NSTRIP):
        xs = xstrip.tile([128, 4, D], F32R, tag="xs",
                         name=f"xs_{x_dram.name}_{s}")
        r0 = s * 512
        if s == 0 and first:
            # separate sub-DMAs for the first strip: lower first-tile latency
            for tt in range(4):
                nc.sync.dma_start(
                    out=xs[:, tt, :],
                    in_=x[r0 + tt * 128:r0 + (tt + 1) * 128, :])
        else:
            nc.sync.dma_start(
                out=xs[:],
                in_=x[r0:r0 + 512, :].rearrange("(tt p) d -> p tt d", p=128))
        for dc in range(NDC):
            pst = ps.tile([128, 512], F32R, tag="ps")
            for tt in range(4):
                nc.tensor.transpose(
                    pst[:, tt * 128:(tt + 1) * 128],
                    xs[:, tt, dc * 128:(dc + 1) * 128], identr)
            nc.vector.tensor_copy(xT[dc][:, r0:r0 + 512], pst[:])
        # projection chunk s: projT[u, r0:r0+512] = sum_dc W[dc].T @ xT[dc]
        for uc in range(2):
            pp = psb.tile([128, 512], F32, tag="psb")
            for dc in range(NDC):
                nc.tensor.matmul(
                    pp[:], w_sb[:, dc, uc * 128:(uc + 1) * 128],
                    xT[dc][:, r0:r0 + 512],
                    start=(dc == 0), stop=(dc == NDC - 1))
            nc.scalar.add(projT[:, uc, r0:r0 + 512], pp[:],
                          add=b_col[:, uc:uc + 1])


def build():
    nc = bacc.Bacc("TRN2", target_bir_lowering=False, debug=False,
                   num_devices=NCORES)
    eeg = nc.dram_tensor("eeg", (T, DE), F32, kind="ExternalInput")
    image = nc.dram_tensor("image", (T, DI), F32, kind="ExternalInput")
    W_e = nc.dram_tensor("W_e", (DE, U), F32, kind="ExternalInput")
    b_e = nc.dram_tensor("b_e", (U,), F32, kind="ExternalInput")
    W_i = nc.dram_tensor("W_i", (DI, U), F32, kind="ExternalInput")
    b_i = nc.dram_tensor("b_i", (U,), F32, kind="ExternalInput")
    att_e = nc.dram_tensor("att_e", (T, U), F32, kind="ExternalOutput")
    att_i = nc.dram_tensor("att_i", (T, U), F32, kind="ExternalOutput")

    with ExitStack() as ctx:
        tc = ctx.enter_context(tile.TileContext(nc))
        const = ctx.enter_context(tc.tile_pool(name="const", bufs=1))
        persist = ctx.enter_context(tc.tile_pool(name="persist", bufs=1))
        xstrip = ctx.enter_context(tc.tile_pool(name="xstrip", bufs=2))
        xt = ctx.enter_context(tc.tile_pool(name="xt", bufs=6))
        ps = ctx.enter_context(tc.tile_pool(name="ps", bufs=6, space="PSUM"))
        psb = ctx.enter_context(tc.tile_pool(name="psb", bufs=2, space="PSUM"))
        small = ctx.enter_context(tc.tile_pool(name="small", bufs=4))
        etp = ctx.enter_context(tc.tile_pool(name="etp", bufs=3))
        outp = ctx.enter_context(tc.tile_pool(name="outp", bufs=2))

        ident = const.tile([128, 128], F32)
        make_identity(nc, ident[:])
        identr = const.tile([128, 128], F32R)
        nc.vector.tensor_copy(identr[:], ident[:])

        # weights / biases: straight f32r-bitcast DMA loads (bit-identical).
        # Issued on the ACT queue so SP's strip stream gets to HWDGE first.
        w_i_sb = const.tile([128, DI // 128, U], F32R)
        w_e_sb = const.tile([128, DE // 128, U], F32R)
        be_col = const.tile([128, 2], F32)
        bi_col = const.tile([128, 2], F32)
        nc.scalar.dma_start(
            out=w_i_sb[:],
            in_=W_i.ap().bitcast(F32R).rearrange("(c p) u -> p c u", p=128))
        nc.scalar.dma_start(
            out=w_e_sb[:],
            in_=W_e.ap().bitcast(F32R).rearrange("(c p) u -> p c u", p=128))
        nc.scalar.dma_start(out=bi_col[:],
                            in_=b_i.ap().rearrange("(c p) -> p c", p=128))
        nc.scalar.dma_start(out=be_col[:],
                            in_=b_e.ap().rearrange("(c p) -> p c", p=128))

        projTe = persist.tile([128, 2, T], F16, tag="projTe")
        projTi = persist.tile([128, 2, T], F16, tag="projTi")
        proj_e = persist.tile([128, TQ, U], F16, tag="proj_e")
        proj_i = persist.tile([128, TQ, U], F16, tag="proj_i")
        E = persist.tile([128, TQ, T], F16, tag="E")
        rZ = persist.tile([128, TQ], F32, tag="rZ")

        pools = (ps, psb, xstrip, xt, identr)

        # ---- phase 1: image (first), then eeg ----
        _phase1_modality(nc, pools, image, DI, w_i_sb, bi_col, projTi,
                         first=True)
        for uc in range(2):
            nc.scalar.dma_start_transpose(
                proj_i[:, :, uc * 128:(uc + 1) * 128], projTi[:, uc, :])
        _phase1_modality(nc, pools, eeg, DE, w_e_sb, be_col, projTe,
                         first=False)
        for uc in range(2):
            nc.scalar.dma_start_transpose(
                proj_e[:, :, uc * 128:(uc + 1) * 128], projTe[:, uc, :])

        # ---- phase 2: scores + softmax per q-tile, AV-eeg lagging ----
        ets = [None] * TQ

        def emit_scores(qt):
            cm = small.tile([128, 4], F32, tag="cm", name=f"cm_{qt}")
            s_chunks = []
            for nk in range(NK):
                s = ps.tile([128, 512], F32, tag="ps", name=f"s_{qt}_{nk}")
                s_chunks.append(s)
                for uc in range(2):
                    nc.tensor.matmul(
                        s[:],
                        projTe[:, uc, qt * 128:(qt + 1) * 128],
                        projTi[:, uc, nk * 512:(nk + 1) * 512],
                        start=(uc == 0), stop=(uc == 1))
                nc.vector.reduce_max(cm[:, nk:nk + 1], s[:], axis=AX)
            negmax = small.tile([128, 1], F32, tag="negmax", name=f"nm_{qt}")
            nc.vector.tensor_reduce(negmax[:], cm[:], axis=AX,
                                    op=mybir.AluOpType.max, negate=True)
            zp = small.tile([128, 4], F32, tag="zp", name=f"zp_{qt}")
            for nk in range(NK):
                nc.scalar.activation(
                    E[:, qt, nk * 512:(nk + 1) * 512], s_chunks[nk][:], EXP,
                    bias=negmax[:], scale=1.0, accum_out=zp[:, nk:nk + 1])
            # one XBAR instruction: all 16 E^T tiles for this q-row
            ett = etp.tile([128, TQ, 128], F16, tag="ett", name=f"ett_{qt}")
            nc.sync.dma_start_transpose(ett[:], E[:, qt, :])
            ets[qt] = ett
            zrow = small.tile([128, 1], F32, tag="zrow", name=f"zr_{qt}")
            nc.vector.reduce_sum(zrow[:], zp[:], axis=AX)
            nc.vector.reciprocal(rZ[:, qt:qt + 1], zrow[:])
            nc.vector.tensor_scalar_mul(
                proj_i[:, qt, :], proj_i[:, qt, :], rZ[:, qt:qt + 1])

        oe_buf = [None]

        def emit_av_eeg(qt):
            ett = ets[qt]
            pav = psb.tile([128, 512], F32, tag="psb", name=f"pav_{qt}")
            for kc in range(TQ):
                nc.tensor.matmul(pav[:, :U], ett[:, kc, :], proj_e[:, kc, :],
                                 start=(kc == 0), stop=(kc == TQ - 1))
            ets[qt] = None
            if qt % 2 == 0:
                oe_buf[0] = outp.tile([128, 2, U], F32, tag="oute",
                                      name=f"oe_{qt}")
            nc.scalar.activation(oe_buf[0][:, qt % 2, :], pav[:, :U], COPY,
                                 scale=rZ[:, qt:qt + 1])
            if qt % 2 == 1:
                q0 = (qt - 1) * 128
                nc.sync.dma_start(
                    out=att_e.ap()[q0:q0 + 256, :].rearrange(
                        "(c p) u -> p c u", p=128),
                    in_=oe_buf[0][:])

        for qt in range(TQ):
            emit_scores(qt)
            if qt >= AV_LAG:
                emit_av_eeg(qt - AV_LAG)
        for qt in range(TQ - AV_LAG, TQ):
            emit_av_eeg(qt)

        # ---- phase 3: att_img[kt] = sum_q E[q, kt-block].T @ (proj_i/Z)[q]
        oi_buf = None
        for kt in range(TQ):
            pav = psb.tile([128, 512], F32, tag="psb", name=f"pvi_{kt}")
            for qc in range(TQ):
                nc.tensor.matmul(
                    pav[:, :U], E[:, qc, kt * 128:(kt + 1) * 128],
                    proj_i[:, qc, :],
                    start=(qc == 0), stop=(qc == TQ - 1))
            if kt % 2 == 0:
                oi_buf = outp.tile([128, 2, U], F32, tag="outi",
                                   name=f"oi_{kt}")
            nc.scalar.copy(oi_buf[:, kt % 2, :], pav[:, :U])
            if kt % 2 == 1:
                k0 = (kt - 1) * 128
                nc.sync.dma_start(
                    out=att_i.ap()[k0:k0 + 256, :].rearrange(
                        "(c p) u -> p c u", p=128),
                    in_=oi_buf[:])

    nc.finalize()
    return nc


_NC_CACHE = {}


def kernel(eeg, image, W_e, b_e, W_i, b_i):
    key = "v2"
    if key not in _NC_CACHE:
        _NC_CACHE[key] = build()
    nc = _NC_CACHE[key]
    eeg = np.ascontiguousarray(eeg, dtype=np.float32)
    image = np.ascontiguousarray(image, dtype=np.float32)
    in_maps = [{
        "eeg": eeg[b], "image": image[b],
        "W_e": np.asarray(W_e, np.float32), "b_e": np.asarray(b_e, np.float32),
        "W_i": np.asarray(W_i, np.float32), "b_i": np.asarray(b_i, np.float32),
    } for b in range(B)]
    res = run_bass_kernel_spmd(nc, in_maps, list(range(NCORES)))
    att_e = np.stack([np.asarray(r["att_e"]) for r in res.results])
    att_i = np.stack([np.asarray(r["att_i"]) for r in res.results])
    return att_e, att_i


# revision 11
# speedup vs baseline: 1.1076x; 1.1076x over previous
"""CrossModalAttention Trainium2 kernel.

Per-core computation (data-parallel over batch, 1 sample per core):
  eeg_proj   = eeg @ W_e + b_e                  [T, U]
  image_proj = image @ W_i + b_i                [T, U]
  scores     = eeg_proj @ image_proj.T          [T, T]
  attn       = softmax(scores, axis=-1)
  att_eeg    = attn @ eeg_proj                  [T, U]
  att_img    = attn.T @ image_proj              [T, U]

Engine split: PE does only real matmuls (proj accumulation in f32r with
fp32 PSUM, scores + AV in fp16) plus the x-transposes (f32r, 1.5 c/row).
All other transposes run on the DMA XBAR (16x128 tile transpose):
projT->proj and the per-row-block E^T needed for att_eeg. Softmax max on
DVE, exp on ACT (exact per-row max subtraction), fp16 attention weights.
Phase 1 is software-pipelined per 512-row strip; phase 2 pipelines
scores(qt) / softmax(qt) / E^T-XBAR(qt) / AV-eeg(qt-3).
End-to-end absmax relative error ~5e-3.
"""
import numpy as np
from contextlib import ExitStack

import concourse.bass as bass
import concourse.bacc as bacc
import concourse.mybir as mybir
import concourse.tile as tile
from concourse.bass_utils import run_bass_kernel_spmd
from concourse.masks import make_identity

F32 = mybir.dt.float32
F32R = mybir.dt.float32r
F16 = mybir.dt.float16
AX = mybir.AxisListType.X
EXP = mybir.ActivationFunctionType.Exp
COPY = mybir.ActivationFunctionType.Copy

B, T, DE, DI, U = 8, 2048, 512, 768, 256
NCORES = 8
TQ = T // 128           # 16 q/k tiles of 128
NK = T // 512           # 4 score chunks of 512
NSTRIP = T // 512       # 4 strips of 512 rows per modality
AV_LAG = 3              # AV-eeg trails scores by 3 q-tiles (XBAR latency)


def build():
    nc = bacc.Bacc("TRN2", target_bir_lowering=False, debug=False,
                   num_devices=NCORES)
    eeg = nc.dram_tensor("eeg", (T, DE), F32, kind="ExternalInput")
    image = nc.dram_tensor("image", (T, DI), F32, kind="ExternalInput")
    W_e = nc.dram_tensor("W_e", (DE, U), F32, kind="ExternalInput")
    b_e = nc.dram_tensor("b_e", (U,), F32, kind="ExternalInput")
    W_i = nc.dram_tensor("W_i", (DI, U), F32, kind="ExternalInput")
    b_i = nc.dram_tensor("b_i", (U,), F32, kind="ExternalInput")
    att_e = nc.dram_tensor("att_e", (T, U), F32, kind="ExternalOutput")
    att_i = nc.dram_tensor("att_i", (T, U), F32, kind="ExternalOutput")

    with ExitStack() as ctx:
        tc = ctx.enter_context(tile.TileContext(nc))
        const = ctx.enter_context(tc.tile_pool(name="const", bufs=1))
        persist = ctx.enter_context(tc.tile_pool(name="persist", bufs=1))
        xstrip = ctx.enter_context(tc.tile_pool(name="xstrip", bufs=2))
        xt = ctx.enter_context(tc.tile_pool(name="xt", bufs=6))
        ps = ctx.enter_context(tc.tile_pool(name="ps", bufs=6, space="PSUM"))
        psb = ctx.enter_context(tc.tile_pool(name="psb", bufs=2, space="PSUM"))
        small = ctx.enter_context(tc.tile_pool(name="small", bufs=4))
        etp = ctx.enter_context(tc.tile_pool(name="etp", bufs=4))
        outp = ctx.enter_context(tc.tile_pool(name="outp", bufs=2))

        ident = const.tile([128, 128], F32)
        make_identity(nc, ident[:])
        identr = const.tile([128, 128], F32R)
        nc.vector.tensor_copy(identr[:], ident[:])

        # weights / biases: straight f32r-bitcast DMA loads (bit-identical).
        # Issued on the ACT queue, before the strip stream on SP.
        w_i_sb = const.tile([128, DI // 128, U], F32R)
        w_e_sb = const.tile([128, DE // 128, U], F32R)
        be_col = const.tile([128, 2], F32)
        bi_col = const.tile([128, 2], F32)
        nc.scalar.dma_start(
            out=w_i_sb[:],
            in_=W_i.ap().bitcast(F32R).rearrange("(c p) u -> p c u", p=128))
        nc.scalar.dma_start(
            out=w_e_sb[:],
            in_=W_e.ap().bitcast(F32R).rearrange("(c p) u -> p c u", p=128))
        nc.scalar.dma_start(out=bi_col[:],
                            in_=b_i.ap().rearrange("(c p) -> p c", p=128))
        nc.scalar.dma_start(out=be_col[:],
                            in_=b_e.ap().rearrange("(c p) -> p c", p=128))

        projTe = persist.tile([128, 2, T], F16, tag="projTe")
        projTi = persist.tile([128, 2, T], F16, tag="projTi")
        proj_e = persist.tile([128, TQ, U], F16, tag="proj_e")
        proj_i = persist.tile([128, TQ, U], F16, tag="proj_i")
        E = persist.tile([128, TQ, T], F16, tag="E")
        rZ = persist.tile([128, TQ], F32, tag="rZ")

        # ---- phase 1 (software-pipelined per strip):
        # step(k): transpose strip k, then projection of strip k-1.
        # Strip order: image strips 0..3, then eeg strips 0..3.
        MODS = [
            dict(x=image, D=DI, w=w_i_sb, b=bi_col, projT=projTi, tiles=None),
            dict(x=eeg, D=DE, w=w_e_sb, b=be_col, projT=projTe, tiles=None),
        ]
        for m in MODS:
            m["tiles"] = [
                xt.tile([128, T], F32R, tag="xt",
                        name=f"xT_{m['x'].name}_{dc}", uniquify=True)
                for dc in range(m["D"] // 128)]

        def emit_load_xp(mi, s):
            m = MODS[mi]
            D = m["D"]
            x = m["x"].ap().bitcast(F32R)
            r0 = s * 512
            xs = xstrip.tile([128, 4, DI], F32R, tag="xs",
                             name=f"xs_{m['x'].name}_{s}")
            nc.sync.dma_start(
                out=xs[:, :, :D],
                in_=x[r0:r0 + 512, :].rearrange("(tt p) d -> p tt d", p=128))
            for dc in range(D // 128):
                pst = ps.tile([128, 512], F32R, tag="ps")
                for tt in range(4):
                    nc.tensor.transpose(
                        pst[:, tt * 128:(tt + 1) * 128],
                        xs[:, tt, dc * 128:(dc + 1) * 128], identr)
                nc.vector.tensor_copy(m["tiles"][dc][:, r0:r0 + 512], pst[:])

        def emit_proj(mi, s):
            m = MODS[mi]
            D = m["D"]
            r0 = s * 512
            for uc in range(2):
                pp = psb.tile([128, 512], F32, tag="psb")
                for dc in range(D // 128):
                    nc.tensor.matmul(
                        pp[:], m["w"][:, dc, uc * 128:(uc + 1) * 128],
                        m["tiles"][dc][:, r0:r0 + 512],
                        start=(dc == 0), stop=(dc == D // 128 - 1))
                nc.scalar.add(m["projT"][:, uc, r0:r0 + 512], pp[:],
                              add=m["b"][:, uc:uc + 1])

        # ---- phase 2 emitters ----
        ets = [None] * TQ
        finz = [None] * TQ   # deferred (zp, ) per qt

        def emit_scores(qt):
            cm = small.tile([128, 4], F32, tag="cm", name=f"cm_{qt}")
            s_chunks = []
            for nk in range(NK):
                s = ps.tile([128, 512], F32, tag="ps", name=f"s_{qt}_{nk}")
                s_chunks.append(s)
                for uc in range(2):
                    nc.tensor.matmul(
                        s[:],
                        projTe[:, uc, qt * 128:(qt + 1) * 128],
                        projTi[:, uc, nk * 512:(nk + 1) * 512],
                        start=(uc == 0), stop=(uc == 1))
                nc.vector.reduce_max(cm[:, nk:nk + 1], s[:], axis=AX)
            negmax = small.tile([128, 1], F32, tag="negmax", name=f"nm_{qt}")
            nc.vector.tensor_reduce(negmax[:], cm[:], axis=AX,
                                    op=mybir.AluOpType.max, negate=True)
            zp = small.tile([128, 4], F32, tag="zp", name=f"zp_{qt}")
            for nk in range(NK):
                nc.scalar.activation(
                    E[:, qt, nk * 512:(nk + 1) * 512], s_chunks[nk][:], EXP,
                    bias=negmax[:], scale=1.0, accum_out=zp[:, nk:nk + 1])
            # one XBAR instruction: all 16 E^T tiles for this q-row
            ett = etp.tile([128, TQ, 128], F16, tag="ett", name=f"ett_{qt}")
            nc.sync.dma_start_transpose(ett[:], E[:, qt, :])
            ets[qt] = ett
            finz[qt] = zp

        def emit_finalize_z(qt):
            # deferred one iteration so DVE's wait-queue isn't clogged in
            # front of the next qt's reduce_max chain
            zp = finz[qt]
            zrow = small.tile([128, 1], F32, tag="zrow", name=f"zr_{qt}")
            nc.vector.reduce_sum(zrow[:], zp[:], axis=AX)
            nc.vector.reciprocal(rZ[:, qt:qt + 1], zrow[:])
            nc.vector.tensor_scalar_mul(
                proj_i[:, qt, :], proj_i[:, qt, :], rZ[:, qt:qt + 1])

        oe_buf = [None]

        def emit_av_eeg(qt):
            ett = ets[qt]
            pav = psb.tile([128, 512], F32, tag="psb", name=f"pav_{qt}")
            for kc in range(TQ):
                nc.tensor.matmul(pav[:, :U], ett[:, kc, :], proj_e[:, kc, :],
                                 start=(kc == 0), stop=(kc == TQ - 1))
            ets[qt] = None
            if qt % 2 == 0:
                oe_buf[0] = outp.tile([128, 2, U], F32, tag="oute",
                                      name=f"oe_{qt}")
            nc.scalar.activation(oe_buf[0][:, qt % 2, :], pav[:, :U], COPY,
                                 scale=rZ[:, qt:qt + 1])
            if qt % 2 == 1:
                q0 = (qt - 1) * 128
                nc.sync.dma_start(
                    out=att_e.ap()[q0:q0 + 256, :].rearrange(
                        "(c p) u -> p c u", p=128),
                    in_=oe_buf[0][:])

        # ---- emission schedule ----
        # phase 1 pipeline: steps over 8 strips (img 0..3 then eeg 0..3)
        strips = [(0, s) for s in range(NSTRIP)] + \
                 [(1, s) for s in range(NSTRIP)]
        for k, (mi, s) in enumerate(strips):
            emit_load_xp(mi, s)
            if k > 0:
                emit_proj(*strips[k - 1])
            if (mi, s) == (0, NSTRIP - 1):
                pass
        # image projT -> proj via XBAR (ACT queue) right when projTi complete
        # is emitted inside the loop order below instead:
        emit_proj(*strips[-1])  # eeg strip 3 projection

        for uc in range(2):
            nc.scalar.dma_start_transpose(
                proj_i[:, :, uc * 128:(uc + 1) * 128], projTi[:, uc, :])
        for uc in range(2):
            nc.scalar.dma_start_transpose(
                proj_e[:, :, uc * 128:(uc + 1) * 128], projTe[:, uc, :])

        # phase 2: scores/softmax per qt; deferred Z; AV-eeg lagging
        for qt in range(TQ):
            emit_scores(qt)
            if qt >= 1:
                emit_finalize_z(qt - 1)
            if qt >= AV_LAG:
                emit_av_eeg(qt - AV_LAG)
        emit_finalize_z(TQ - 1)
        for qt in range(TQ - AV_LAG, TQ):
            emit_av_eeg(qt)

        # ---- phase 3: att_img[kt] = sum_q E[q, kt-block].T @ (proj_i/Z)[q]
        oi_buf = None
        for kt in range(TQ):
            pav = psb.tile([128, 512], F32, tag="psb", name=f"pvi_{kt}")
            for qc in range(TQ):
                nc.tensor.matmul(
                    pav[:, :U], E[:, qc, kt * 128:(kt + 1) * 128],
                    proj_i[:, qc, :],
                    start=(qc == 0), stop=(qc == TQ - 1))
            if kt % 2 == 0:
                oi_buf = outp.tile([128, 2, U], F32, tag="outi",
                                   name=f"oi_{kt}")
            nc.scalar.copy(oi_buf[:, kt % 2, :], pav[:, :U])
            if kt % 2 == 1:
                k0 = (kt - 1) * 128
                nc.sync.dma_start(
                    out=att_i.ap()[k0:k0 + 256, :].rearrange(
                        "(c p) u -> p c u", p=128),
                    in_=oi_buf[:])

    nc.finalize()
    return nc


_NC_CACHE = {}


def kernel(eeg, image, W_e, b_e, W_i, b_i):
    key = "v2"
    if key not in _NC_CACHE:
        _NC_CACHE[key] = build()
    nc = _NC_CACHE[key]
    eeg = np.ascontiguousarray(eeg, dtype=np.float32)
    image = np.ascontiguousarray(image, dtype=np.float32)
    in_maps = [{
        "eeg": eeg[b], "image": image[b],
        "W_e": np.asarray(W_e, np.float32), "b_e": np.asarray(b_e, np.float32),
        "W_i": np.asarray(W_i, np.float32), "b_i": np.asarray(b_i, np.float32),
    } for b in range(B)]
    res = run_bass_kernel_spmd(nc, in_maps, list(range(NCORES)))
    att_e = np.stack([np.asarray(r["att_e"]) for r in res.results])
    att_i = np.stack([np.asarray(r["att_i"]) for r in res.results])
    return att_e, att_i
